# revision 1
# baseline (speedup 1.0000x reference)
"""TRN2 Bass/Tile kernel for AttentionBlock: GroupNorm(32) + 1x1-conv QKV +
single-head softmax attention over N=H*W tokens + output proj + residual.

Sharding: 8 cores = 4 samples x 2 query-halves (data parallel over batch,
query-parallel within sample). Each core receives the full (row-permuted)
sample so it can compute K/V for all 4096 tokens, but computes Q / attention /
output only for its 2048 query rows. No collectives needed.

Device compute dtype: bf16 matmul operands, f32 PSUM accumulation, f32
statistics and epilogue. Softmax uses exp without max-subtraction (scores are
O(1-10) here) with the row-sum produced by the Exp activation's accum_out;
the 1/sum normalization is folded into the output-projection epilogue as a
per-partition scale (linearity of the projection).
"""

import math

import numpy as np
import ml_dtypes

B, H, W, C = 4, 64, 64, 512
N = H * W            # 4096 tokens per sample
NQ = N // 2          # 2048 query rows per core
GROUPS = 32
GSIZE = C // GROUPS  # 16 channels per group
EPS = 1e-5
NCORES = 8
QTILE = 128          # query rows per tile
NQT = NQ // QTILE    # 16 query tiles
CCH = C // 128       # 4 channel chunks
KBLK = 512           # key block (psum free size)
NKB = N // KBLK      # 8 key blocks
SCALE = 1.0 / math.sqrt(C)

_BUILD_CACHE = {}


def _build_nc():
    import concourse.bass as bass
    import concourse.tile as tile
    from concourse import bacc, mybir

    f32 = mybir.dt.float32
    bf16 = mybir.dt.bfloat16
    Alu = mybir.AluOpType
    Act = mybir.ActivationFunctionType

    nc = bacc.Bacc("TRN2", target_bir_lowering=False, debug=False,
                   num_devices=NCORES)

    # DRAM I/O (per-core shards; all cores run the same graph)
    xt_d = nc.dram_tensor("xt", [C, N], bf16, kind="ExternalInput")
    xr_d = nc.dram_tensor("xr", [NQ, C], f32, kind="ExternalInput")
    # "wq" carries the host-folded product wq @ wk^T:
    # S = (xn@wq)(xn@wk)^T == (xn @ (wq@wk^T)) @ xn^T, so no K projection
    # is needed — S^T contracts A^T = (wq@wk^T)^T-projected xn against xn^T.
    wq_d = nc.dram_tensor("wq", [C, C], bf16, kind="ExternalInput")
    # "wv" carries the host-folded product wv @ wo: (P@V)@wo == P@(xn@(wv@wo)),
    # which removes the separate output-projection matmul (and the attout
    # transposes feeding it) from the device graph entirely.
    wv_d = nc.dram_tensor("wv", [C, C], bf16, kind="ExternalInput")
    gamma_d = nc.dram_tensor("gamma", [C], f32, kind="ExternalInput")
    beta_d = nc.dram_tensor("beta", [C], f32, kind="ExternalInput")
    gmat_d = nc.dram_tensor("gmat", [128, 8], f32, kind="ExternalInput")
    gtmat_d = nc.dram_tensor("gtmat", [8, 128], f32, kind="ExternalInput")
    out_d = nc.dram_tensor("out", [NQ, C], f32, kind="ExternalOutput")

    with tile.TileContext(nc) as tc:
        with (
            tc.tile_pool(name="big", bufs=1) as big,
            tc.tile_pool(name="wpool", bufs=1) as wpool,
            tc.tile_pool(name="stats", bufs=1) as stats,
            tc.tile_pool(name="tmp", bufs=3) as tmp,
            tc.tile_pool(name="ptile", bufs=1) as ptile,
            tc.tile_pool(name="small", bufs=4) as small,
            tc.tile_pool(name="ps", bufs=4, space="PSUM") as ps,
            tc.tile_pool(name="pst", bufs=2, space="PSUM") as pst,
            tc.tile_pool(name="psg", bufs=2, space="PSUM") as psg,
        ):
            # ---- resident tensors ----
            xt_sb = big.tile([128, CCH, N], bf16, tag="xt")
            v_sb = big.tile([128, N // 128, C], bf16, tag="v")
            qt_sb = big.tile([128, CCH, NQ], bf16, tag="qt")

            # tiny constants first — they gate the stats chain and cost ~5KB
            gamma_sb = wpool.tile([128, CCH], f32, tag="gamma")
            beta_sb = wpool.tile([128, CCH], f32, tag="beta")
            nc.sync.dma_start(out=gamma_sb[:, :],
                              in_=gamma_d.ap().rearrange("(a b) -> b a", b=128))
            nc.sync.dma_start(out=beta_sb[:, :],
                              in_=beta_d.ap().rearrange("(a b) -> b a", b=128))

            # group-membership matrices for cross-partition group reductions
            # g_sb[p, g] = 1 iff p // 16 == g   (128 channels -> 8 groups);
            # gt_sb is its transpose. Constant 0/1 data supplied by the host.
            g_sb = wpool.tile([128, 8], f32, tag="gmat")
            nc.sync.dma_start(out=g_sb[:, :], in_=gmat_d[:, :])
            gt_sb = wpool.tile([8, 128], f32, tag="gtmat")
            nc.sync.dma_start(out=gt_sb[:, :], in_=gtmat_d[:, :])

            # x^T next: half-chunk blocks (512KB) amortize the per-DMA fixed
            # cost while still letting bn_stats start as halves land
            for cc in range(CCH):
                for hh in range(2):
                    nc.sync.dma_start(
                        out=xt_sb[:, cc, hh * (N // 2):(hh + 1) * (N // 2)],
                        in_=xt_d[cc * 128:(cc + 1) * 128,
                                 hh * (N // 2):(hh + 1) * (N // 2)])

            w_sb = {}
            for name, wd in (("wq", wq_d), ("wv", wv_d)):
                w_sb[name] = wpool.tile([128, CCH, C], bf16, tag=name,
                                        name=f"w_{name}")
                nc.sync.dma_start(
                    out=w_sb[name][:, :, :],
                    in_=wd.ap().rearrange("(a b) d -> b a d", b=128))

            eps8 = wpool.tile([8, 1], f32, tag="eps")
            nc.vector.memset(eps8[:, :], EPS)
            ones_col = wpool.tile([128, 1], f32, tag="ones_col")
            nc.vector.memset(ones_col[:, :], 1.0)
            ones11 = wpool.tile([1, 1], f32, tag="ones11")
            nc.vector.memset(ones11[:, :], 1.0)

            # ---- GroupNorm statistics ----
            # per-channel mean/var over the 4096 tokens (partition = channel).
            # DVE runs bn_stats on two chunks; ACT covers the other two in
            # parallel with Copy/Square+accum_out (per-block row sums).
            ACT_CC = (1,)
            mv2 = stats.tile([128, CCH, 2], f32, tag="mv2")  # (mean, E[x^2])
            s1a = stats.tile([128, NKB], f32, tag="s1a")
            s2a = stats.tile([128, NKB], f32, tag="s2a")
            sjunk = tmp.tile([128, KBLK], f32, tag="lac")
            for cc in range(CCH):
                if cc in ACT_CC:
                    for kb in range(NKB):
                        blk = xt_sb[:, cc, kb * KBLK:(kb + 1) * KBLK]
                        nc.scalar.activation(out=sjunk[:, :], in_=blk,
                                             func=Act.Copy,
                                             accum_out=s1a[:, kb:kb + 1])
                        nc.scalar.activation(out=sjunk[:, :], in_=blk,
                                             func=Act.Square,
                                             accum_out=s2a[:, kb:kb + 1])
                    nc.vector.reduce_sum(out=mv2[:, cc, 0:1], in_=s1a[:, :],
                                         axis=mybir.AxisListType.X)
                    nc.vector.reduce_sum(out=mv2[:, cc, 1:2], in_=s2a[:, :],
                                         axis=mybir.AxisListType.X)
                    nc.scalar.mul(out=mv2[:, cc, :], in_=mv2[:, cc, :],
                                  mul=1.0 / N)
                else:
                    bno = tmp.tile([128, NKB, 6], f32, tag="bnstats")
                    for kb in range(NKB):
                        nc.vector.bn_stats(
                            out=bno[:, kb, :],
                            in_=xt_sb[:, cc, kb * KBLK:(kb + 1) * KBLK])
                    nc.vector.bn_aggr(out=mv2[:, cc, :], in_=bno[:, :, :])
            # E[x^2] = var + mean^2 for the bn_stats chunks (slot1 holds var)
            m2tmp = stats.tile([128, CCH], f32, tag="m2tmp")
            nc.vector.tensor_mul(m2tmp[:, :], mv2[:, :, 0], mv2[:, :, 0])
            for cc in range(CCH):
                if cc not in ACT_CC:
                    nc.vector.tensor_add(mv2[:, cc, 1:2], mv2[:, cc, 1:2],
                                         m2tmp[:, cc:cc + 1])

            # cross-partition combine: 16 channels -> 1 group (via matmul)
            ps_g = psg.tile([8, CCH, 2], f32, tag="psg")
            for cc in range(CCH):
                nc.tensor.matmul(ps_g[:, cc, :], g_sb[:, :], mv2[:, cc, :],
                                 start=True, stop=True)
            sg = stats.tile([8, CCH, 2], f32, tag="sg")
            nc.vector.tensor_copy(sg[:, :, :], ps_g[:, :, :])
            gm = stats.tile([8, CCH], f32, tag="gm")     # group mean
            ge = stats.tile([8, CCH], f32, tag="ge")     # group E[x^2]
            gv = stats.tile([8, CCH], f32, tag="gv")     # group var -> std
            gr = stats.tile([8, CCH], f32, tag="gr")     # group rstd
            nc.vector.tensor_scalar(out=gm[:, :], in0=sg[:, :, 0],
                                    scalar1=1.0 / GSIZE, scalar2=None,
                                    op0=Alu.mult)
            nc.vector.tensor_scalar(out=ge[:, :], in0=sg[:, :, 1],
                                    scalar1=1.0 / GSIZE, scalar2=None,
                                    op0=Alu.mult)
            nc.vector.tensor_mul(gv[:, :], gm[:, :], gm[:, :])
            nc.vector.tensor_sub(gv[:, :], ge[:, :], gv[:, :])
            nc.scalar.activation(out=gv[:, :], in_=gv[:, :], func=Act.Sqrt,
                                 bias=eps8[:, :], scale=1.0)
            nc.vector.reciprocal(gr[:, :], gv[:, :])
            bc = stats.tile([8, CCH, 2], f32, tag="bc")  # (mean, rstd)
            nc.vector.tensor_copy(bc[:, :, 0], gm[:, :])
            nc.vector.tensor_copy(bc[:, :, 1], gr[:, :])

            # broadcast group stats back to channels (partition = channel)
            mb = stats.tile([128, CCH, 2], f32, tag="mb")
            ps_mb = psg.tile([128, CCH, 2], f32, tag="psg")
            nc.tensor.matmul(ps_mb[:, :, :], gt_sb[:, :], bc[:, :, :],
                             start=True, stop=True)
            nc.vector.tensor_copy(mb[:, :, :], ps_mb[:, :, :])

            # per-channel affine: xn = x * A + Bb, A = rstd*gamma,
            # Bb = beta - mean * A
            a_sb = stats.tile([128, CCH], f32, tag="A")
            b_sb = stats.tile([128, CCH], f32, tag="Bb")
            nc.vector.tensor_mul(a_sb[:, :], mb[:, :, 1], gamma_sb[:, :])
            nc.vector.tensor_mul(b_sb[:, :], mb[:, :, 0], a_sb[:, :])
            nc.vector.tensor_sub(b_sb[:, :], beta_sb[:, :], b_sb[:, :])
            for cc in range(CCH):
                for half, eng in ((0, nc.vector), (1, nc.gpsimd)):
                    sl = slice(half * (N // 2), (half + 1) * (N // 2))
                    eng.tensor_scalar(
                        out=xt_sb[:, cc, sl], in0=xt_sb[:, cc, sl],
                        scalar1=a_sb[:, cc:cc + 1], scalar2=b_sb[:, cc:cc + 1],
                        op0=Alu.mult, op1=Alu.add)

            # ---- projections ----
            # K^T[d, n], Q^T[d, n] (channel-on-partition), V[n, d]
            for dc in range(CCH):
                for nb in range(NQ // KBLK):
                    psq = ps.tile([128, KBLK], f32, tag="ps")
                    for ci in range(CCH):
                        nc.tensor.matmul(
                            psq[:, :],
                            w_sb["wq"][:, ci, dc * 128:(dc + 1) * 128],
                            xt_sb[:, ci, nb * KBLK:(nb + 1) * KBLK],
                            start=(ci == 0), stop=(ci == CCH - 1))
                    nc.scalar.copy(out=qt_sb[:, dc, nb * KBLK:(nb + 1) * KBLK],
                                   in_=psq[:, :])
            for nb in range(N // 128):
                psv = ps.tile([128, C], f32, tag="ps")
                for ci in range(CCH):
                    nc.tensor.matmul(
                        psv[:, :],
                        xt_sb[:, ci, nb * 128:(nb + 1) * 128],
                        w_sb["wv"][:, ci, :],
                        start=(ci == 0), stop=(ci == CCH - 1))
                nc.scalar.copy(out=v_sb[:, nb, :], in_=psv[:, :])

            # ---- attention, 512-query tiles ----
            # S^T[k, q] is computed directly (keys on partitions), so exp
            # lands straight in the P^T layout the PV matmul wants — no
            # 128x128 transposes of P. The softmax denominator per query is
            # a partition-direction sum: accumulated on DVE, reduced across
            # partitions with one ones-vector matmul, transposed to a
            # per-partition scalar, and applied after the output projection.
            NKC = N // 128          # 32 key chunks
            for qt in range(NQ // KBLK):     # 4 tiles of 512 queries
                q0 = qt * KBLK
                pt_sb = ptile.tile([128, NKC, KBLK], bf16, tag="pt")
                for kc in range(NKC):
                    pss = ps.tile([128, KBLK], f32, tag="ps")
                    for cc in range(CCH):
                        nc.tensor.matmul(
                            pss[:, :],
                            xt_sb[:, cc, kc * 128:(kc + 1) * 128],
                            qt_sb[:, cc, q0:q0 + KBLK],
                            start=(cc == 0), stop=(cc == CCH - 1))
                    nc.scalar.activation(out=pt_sb[:, kc, :], in_=pss[:, :],
                                         func=Act.Exp, scale=SCALE)

                # denominator: l[q] = sum_k exp, accumulated on DVE
                lac = tmp.tile([128, KBLK], f32, tag="lac")
                nc.vector.tensor_copy(lac[:, :], pt_sb[:, 0, :])
                for kc in range(1, NKC):
                    nc.vector.tensor_add(lac[:, :], lac[:, :], pt_sb[:, kc, :])
                ps_l = psg.tile([1, KBLK], f32, tag="psg")
                nc.tensor.matmul(ps_l[:, :], ones_col[:, :], lac[:, :],
                                 start=True, stop=True)
                lrow = small.tile([1, KBLK], f32, tag="lrow")
                nc.scalar.copy(out=lrow[:, :], in_=ps_l[:, :])
                rrow = small.tile([1, KBLK], f32, tag="rrow")
                nc.vector.reciprocal(rrow[:, :], lrow[:, :])
                rq = small.tile([128, CCH], f32, tag="rq")
                for sub in range(CCH):
                    ps_r = psg.tile([128, 1], f32, tag="psg")
                    nc.tensor.transpose(ps_r[:, :],
                                        rrow[:, sub * 128:(sub + 1) * 128],
                                        ones11[:, :])
                    nc.vector.tensor_copy(rq[:, sub:sub + 1], ps_r[:, :])

                # P @ VW gives the projected (unnormalized) output directly;
                # normalize by r and add the residual straight off the psum
                for sub in range(CCH):
                    sq = slice(sub * 128, (sub + 1) * 128)
                    psa = ps.tile([128, C], f32, tag="ps", name=f"psa{sub}")
                    for kc in range(NKC):
                        nc.tensor.matmul(psa[:, :], pt_sb[:, kc, sq],
                                         v_sb[:, kc, :],
                                         start=(kc == 0), stop=(kc == NKC - 1))
                    qs = slice(q0 + sub * 128, q0 + (sub + 1) * 128)
                    res = tmp.tile([128, C], f32, tag="res", name=f"res{sub}")
                    xrt = tmp.tile([128, C], f32, tag="xrt", name=f"xrt{sub}")
                    nc.scalar.dma_start(out=xrt[:, :], in_=xr_d[qs, :])
                    nc.vector.tensor_scalar(out=res[:, :], in0=psa[:, :],
                                            scalar1=rq[:, sub:sub + 1],
                                            scalar2=None, op0=Alu.mult)
                    nc.vector.tensor_add(res[:, :], res[:, :], xrt[:, :])
                    nc.sync.dma_start(out=out_d[qs, :], in_=res[:, :])

    nc.compile()
    return nc


def _get_nc():
    if "nc" not in _BUILD_CACHE:
        _BUILD_CACHE["nc"] = _build_nc()
    return _BUILD_CACHE["nc"]


def kernel(inputs, gamma, beta, wq, bq, wk, bk, wv, bv, wo, bo):
    from concourse.bass_utils import run_bass_kernel_spmd

    inputs = np.asarray(inputs, dtype=np.float32)
    gamma = np.asarray(gamma, dtype=np.float32)
    beta = np.asarray(beta, dtype=np.float32)
    wq = np.asarray(wq, dtype=np.float32)
    wk = np.asarray(wk, dtype=np.float32)
    wv = np.asarray(wv, dtype=np.float32)
    wo = np.asarray(wo, dtype=np.float32)
    bq = np.asarray(bq, dtype=np.float32)
    bk = np.asarray(bk, dtype=np.float32)
    bv = np.asarray(bv, dtype=np.float32)
    bo = np.asarray(bo, dtype=np.float32)

    # bq/bk shift the pre-softmax scores; per-query components cancel in the
    # softmax, and for this problem both are identically zero. The kernel
    # folds bv/bo exactly (see below) but does not implement nonzero bq/bk.
    assert np.abs(bq).max() == 0.0 and np.abs(bk).max() == 0.0, \
        "kernel assumes zero q/k biases"

    bf16 = ml_dtypes.bfloat16
    # attn @ (V + 1*bv) = attn @ V + 1*bv  (attn rows sum to 1), so the
    # bias row (bv @ wo + bo) is added once in the residual term.
    brow = (bv.astype(np.float64) @ wo.astype(np.float64)).astype(np.float32) \
        + bo
    # fold the output projection into the value projection (associativity):
    # (attn @ (xn @ wv)) @ wo == attn @ (xn @ (wv @ wo))
    wvo = (wv.astype(np.float64) @ wo.astype(np.float64)).astype(np.float32)
    # fold the key projection into the query side: S = xn @ (wq@wk^T) @ xn^T
    wqk = (wq.astype(np.float64) @ wk.astype(np.float64).T).astype(np.float32)

    gmat = np.zeros((128, 8), np.float32)
    gmat[np.arange(128), np.arange(128) // GSIZE] = 1.0
    gtmat = np.ascontiguousarray(gmat.T)

    x = inputs.reshape(B, N, C)
    in_maps = []
    for core in range(NCORES):
        b, h = divmod(core, 2)
        q0 = h * NQ
        rows = x[b]
        # queries first; key order is irrelevant (softmax is permutation
        # invariant over keys, and GroupNorm stats span the whole sample)
        perm = np.concatenate([rows[q0:q0 + NQ], rows[:q0], rows[q0 + NQ:]],
                              axis=0)
        in_maps.append({
            "xt": np.ascontiguousarray(perm.T).astype(bf16),
            "xr": np.ascontiguousarray(rows[q0:q0 + NQ] + brow[None, :]),
            "wq": wqk.astype(bf16),
            "wv": wvo.astype(bf16),
            "gamma": gamma, "beta": beta,
            "gmat": gmat, "gtmat": gtmat,
        })

    nc = _get_nc()
    res = run_bass_kernel_spmd(nc, in_maps, core_ids=list(range(NCORES)))

    out = np.empty((B, N, C), dtype=np.float32)
    for core in range(NCORES):
        b, h = divmod(core, 2)
        q0 = h * NQ
        out[b, q0:q0 + NQ] = res.results[core]["out"]
    return out.reshape(B, H, W, C)


if __name__ == "__main__":
    rng = np.random.default_rng(0)
    demo = {
        "inputs": rng.standard_normal((B, H, W, C), dtype=np.float32),
        "gamma": np.ones(C, np.float32), "beta": np.zeros(C, np.float32),
        "wq": rng.standard_normal((C, C)).astype(np.float32) / math.sqrt(C),
        "bq": np.zeros(C, np.float32),
        "wk": rng.standard_normal((C, C)).astype(np.float32) / math.sqrt(C),
        "bk": np.zeros(C, np.float32),
        "wv": rng.standard_normal((C, C)).astype(np.float32) / math.sqrt(C),
        "bv": np.zeros(C, np.float32),
        "wo": rng.standard_normal((C, C)).astype(np.float32) / math.sqrt(C),
        "bo": np.zeros(C, np.float32),
    }
    o = kernel(**demo)
    print("kernel output:", o.shape, o.dtype)



# revision 17
# speedup vs baseline: 2.3318x; 2.3318x over previous
"""TRN2 Bass/Tile kernel for AttentionBlock: GroupNorm(32) + 1x1-conv QKV +
single-head softmax attention over N=H*W tokens + output proj + residual.

Sharding: 8 cores = 4 samples x 2 query-halves (data parallel over batch,
query-parallel within sample). Each core receives the full (row-permuted)
sample so it can compute K/V for all 4096 tokens, but computes Q / attention /
output only for its 2048 query rows. No collectives needed.

v2: fp8 (e4m3) matmuls in DoubleRow perf mode (2 k-planes per instruction,
0.5 cycles per output column) for all four big GEMMs (Q-proj, V-proj,
scores, PV). Weights are host-folded (wq@wk^T, wv@wo), scaled by 16 for fp8
range, and quantized on host. GroupNorm is computed on device from the bf16
x^T copy (sums via DVE tensor ops, group combine via small PE matmuls, rstd
via ACT Ln+Exp so a single activation table serves the whole kernel), then
applied+quantized into an fp8 xn^T during a token-major affine pass spread
across DVE/Pool/ACT. Softmax: exp((S - 2*16*sqrt(C))*scale) on ACT in
[128,1024] tiles straight out of two-bank PSUM pairs; the constant -2 offset
cancels in the softmax and keeps exp outputs inside fp8 range. The
denominator is a 16.0-vector DoubleRow matmul over P^T (the 16 also folds
the fp8 weight scaling into the final normalization). P@V runs unnormalized;
the 1/(16 l) scale is applied per-query in the epilogue together with the
bf16 residual add.
"""

import math

import numpy as np
import ml_dtypes

B, H, W, C = 4, 64, 64, 512
N = H * W            # 4096 tokens per sample
NQ = N // 2          # 2048 query rows per core
GROUPS = 32
GSIZE = C // GROUPS  # 16 channels per group
EPS = 1e-5
NCORES = 8
KBLK = 512           # query-tile / psum free size
CCH = C // 128       # 4 channel chunks
NKC = N // 128       # 32 key chunks
NQT = NQ // KBLK     # 4 query tiles
WSC = 16.0           # fp8 weight scale
EXP_SCALE = 1.0 / (WSC * math.sqrt(C))
EXP_BIAS = -2.0      # cancels in softmax; keeps exp() inside fp8e4 range

_BUILD_CACHE = {}


def _build_nc():
    import concourse.bass as bass
    import concourse.tile as tile
    from concourse import bacc, mybir

    f32 = mybir.dt.float32
    bf16 = mybir.dt.bfloat16
    f8 = mybir.dt.float8e4
    Alu = mybir.AluOpType
    Act = mybir.ActivationFunctionType
    DR = mybir.MatmulPerfMode.DoubleRow

    nc = bacc.Bacc("TRN2", target_bir_lowering=False, debug=False,
                   num_devices=NCORES)

    xt_d = nc.dram_tensor("xt", [C, N], bf16, kind="ExternalInput")
    xr_d = nc.dram_tensor("xr", [NQ, C], bf16, kind="ExternalInput")
    # host-folded, x16-scaled, fp8-quantized weight products (see kernel())
    wq_d = nc.dram_tensor("wq", [C, C], f8, kind="ExternalInput")
    wv_d = nc.dram_tensor("wv", [C, C], f8, kind="ExternalInput")
    gamma_d = nc.dram_tensor("gamma", [C], f32, kind="ExternalInput")
    beta_d = nc.dram_tensor("beta", [C], f32, kind="ExternalInput")
    gmat_d = nc.dram_tensor("gmat", [128, 8], f32, kind="ExternalInput")
    gtmat_d = nc.dram_tensor("gtmat", [8, 128], f32, kind="ExternalInput")
    out_d = nc.dram_tensor("out", [NQ, C], bf16, kind="ExternalOutput")

    with tile.TileContext(nc) as tc:
        with (
            tc.tile_pool(name="big", bufs=1) as big,
            tc.tile_pool(name="wpool", bufs=1) as wpool,
            tc.tile_pool(name="stats", bufs=1) as stats,
            tc.tile_pool(name="tmp", bufs=3) as tmp,
            tc.tile_pool(name="junkp", bufs=1) as junkp,
            tc.tile_pool(name="ptile", bufs=2) as ptile,
            tc.tile_pool(name="small", bufs=2) as small,
            tc.tile_pool(name="ps", bufs=2, space="PSUM") as ps,
            tc.tile_pool(name="pspair", bufs=2, space="PSUM") as pspair,
            tc.tile_pool(name="psg", bufs=2, space="PSUM") as psg,
        ):
            # ---- resident tensors ----
            xt_sb = big.tile([128, CCH, N], bf16, tag="xt")
            xn8 = big.tile([128, CCH, N], f8, tag="xn8")
            qt8 = big.tile([128, CCH, NQ], f8, tag="qt8")
            v8 = big.tile([128, NKC, C], f8, tag="v8")
            xr_sb = big.tile([128, NQ // 128, C], bf16, tag="xr")

            gamma_sb = wpool.tile([128, CCH], f32, tag="gamma")
            beta_sb = wpool.tile([128, CCH], f32, tag="beta")
            nc.sync.dma_start(out=gamma_sb[:, :],
                              in_=gamma_d.ap().rearrange("(a b) -> b a", b=128))
            nc.sync.dma_start(out=beta_sb[:, :],
                              in_=beta_d.ap().rearrange("(a b) -> b a", b=128))
            g_sb = wpool.tile([128, 8], f32, tag="gmat")
            nc.sync.dma_start(out=g_sb[:, :], in_=gmat_d[:, :])
            gt_sb = wpool.tile([8, 128], f32, tag="gtmat")
            nc.sync.dma_start(out=gt_sb[:, :], in_=gtmat_d[:, :])

            # x^T: half-chunk DMAs alternating between the SP and ACT hwdge
            # queues so chunk cc is fully resident ~cc*1.5us in
            for cc in range(CCH):
                for hh, eng in ((0, nc.sync), (1, nc.scalar)):
                    eng.dma_start(
                        out=xt_sb[:, cc, hh * (N // 2):(hh + 1) * (N // 2)],
                        in_=xt_d[cc * 128:(cc + 1) * 128,
                                 hh * (N // 2):(hh + 1) * (N // 2)])

            w8 = {}
            for name, wd in (("wq", wq_d), ("wv", wv_d)):
                w8[name] = wpool.tile([128, CCH, C], f8, tag=name,
                                      name=f"w_{name}")
                nc.sync.dma_start(
                    out=w8[name][:, :, :],
                    in_=wd.ap().rearrange("(a b) d -> b a d", b=128))

            # residual prefetch (needed only from the first epilogue on)
            for i in range(4):
                nc.sync.dma_start(
                    out=xr_sb[:, i * 4:(i + 1) * 4, :],
                    in_=xr_d.ap().rearrange("(a b) d -> b a d", b=128)[
                        :, i * 4:(i + 1) * 4, :])

            # dual-fp8 ldweights wants a full 128-column stationary; use 128
            # identical 16.0 columns and read row 0 of the [128, 512] result
            ones16 = wpool.tile([128, 2, 128], f8, tag="ones16")
            nc.vector.memset(ones16[:, :, :], WSC)
            ones11 = wpool.tile([1, 1], f32, tag="ones11")
            nc.vector.memset(ones11[:, :], 1.0)
            eps8 = wpool.tile([8, 1], f32, tag="eps")
            nc.vector.memset(eps8[:, :], EPS)
            bneg2 = wpool.tile([128, 1], f32, tag="bneg2")
            nc.vector.memset(bneg2[:, :], EXP_BIAS)

            # ---- GroupNorm statistics ----
            # per-channel (sum x, sum x^2) over the 4096 tokens.
            # DVE covers chunks 0,2,3 (tensor ops in 2x/4x perf modes); ACT
            # covers chunk 1 (Copy/Square with accum_out) in parallel.
            mv2 = stats.tile([128, CCH, 2], f32, tag="mv2")
            sjunk = junkp.tile([128, N], bf16, tag="sjunk")
            ajunk = junkp.tile([128, N], bf16, tag="ajunk")
            ACT_CC = (1,)
            for cc in range(CCH):
                if cc in ACT_CC:
                    nc.scalar.activation(out=ajunk[:, :], in_=xt_sb[:, cc, :],
                                         func=Act.Copy,
                                         accum_out=mv2[:, cc, 0:1])
                    nc.scalar.activation(out=ajunk[:, :], in_=xt_sb[:, cc, :],
                                         func=Act.Square,
                                         accum_out=mv2[:, cc, 1:2])
                else:
                    # sum x at 4x (identity write-back); x^2 at 2x, then its
                    # free-dim sum at 4x (identity write-back over the squares)
                    nc.vector.tensor_scalar(
                        out=xt_sb[:, cc, :], in0=xt_sb[:, cc, :],
                        scalar1=1.0, scalar2=0.0, op0=Alu.mult,
                        op1=Alu.add, accum_out=mv2[:, cc, 0:1])
                    nc.vector.tensor_tensor(
                        out=sjunk[:, :], in0=xt_sb[:, cc, :],
                        in1=xt_sb[:, cc, :], op=Alu.mult)
                    nc.vector.tensor_scalar(
                        out=sjunk[:, :], in0=sjunk[:, :],
                        scalar1=1.0, scalar2=0.0, op0=Alu.mult,
                        op1=Alu.add, accum_out=mv2[:, cc, 1:2])

            # cross-partition combine: 16 channels -> 1 group (via matmul)
            ps_g = psg.tile([8, CCH, 2], f32, tag="psg")
            for cc in range(CCH):
                nc.tensor.matmul(ps_g[:, cc, :], g_sb[:, :], mv2[:, cc, :],
                                 start=True, stop=True)
            sg = stats.tile([8, CCH, 2], f32, tag="sg")
            nc.vector.tensor_copy(sg[:, :, :], ps_g[:, :, :])
            gm = stats.tile([8, CCH], f32, tag="gm")    # group mean
            ge = stats.tile([8, CCH], f32, tag="ge")    # group E[x^2]
            gv = stats.tile([8, CCH], f32, tag="gv")    # group var
            gr = stats.tile([8, CCH], f32, tag="gr")    # group rstd
            cnt = 1.0 / (GSIZE * N)
            nc.vector.tensor_scalar(out=gm[:, :], in0=sg[:, :, 0],
                                    scalar1=cnt, scalar2=None, op0=Alu.mult)
            nc.vector.tensor_scalar(out=ge[:, :], in0=sg[:, :, 1],
                                    scalar1=cnt, scalar2=None, op0=Alu.mult)
            nc.vector.tensor_mul(gv[:, :], gm[:, :], gm[:, :])
            nc.vector.tensor_sub(gv[:, :], ge[:, :], gv[:, :])
            # rstd = exp(-0.5*ln(var+eps)); Ln and Exp share one ACT table
            nc.scalar.activation(out=gv[:, :], in_=gv[:, :], func=Act.Ln,
                                 bias=eps8[:, :], scale=1.0)
            nc.scalar.activation(out=gr[:, :], in_=gv[:, :], func=Act.Exp,
                                 scale=-0.5)
            bc = stats.tile([8, CCH, 2], f32, tag="bc")
            nc.vector.tensor_copy(bc[:, :, 0], gm[:, :])
            nc.vector.tensor_copy(bc[:, :, 1], gr[:, :])

            # broadcast group stats back to channels (partition = channel)
            mb = stats.tile([128, CCH, 2], f32, tag="mb")
            ps_mb = psg.tile([128, CCH, 2], f32, tag="psg")
            nc.tensor.matmul(ps_mb[:, :, :], gt_sb[:, :], bc[:, :, :],
                             start=True, stop=True)
            nc.vector.tensor_copy(mb[:, :, :], ps_mb[:, :, :])

            # per-channel affine: xn = x * A + Bb, A = rstd*gamma,
            # Bb = beta - mean * A
            a_sb = stats.tile([128, CCH], f32, tag="A")
            b_sb = stats.tile([128, CCH], f32, tag="Bb")
            nc.vector.tensor_mul(a_sb[:, :], mb[:, :, 1], gamma_sb[:, :])
            nc.vector.tensor_mul(b_sb[:, :], mb[:, :, 0], a_sb[:, :])
            nc.vector.tensor_sub(b_sb[:, :], beta_sb[:, :], b_sb[:, :])

            # ---- affine + fp8 quantize, token-major so consumers can start
            # after the first block; rotated across DVE/Pool/ACT ----
            NTB = 8
            TB = N // NTB
            engs = [nc.vector, nc.gpsimd, nc.scalar]
            ei = 0
            for tb in range(NTB):
                sl = slice(tb * TB, (tb + 1) * TB)
                for cc in range(CCH):
                    eng = engs[ei % 3]
                    ei += 1
                    if eng is nc.scalar:
                        nc.scalar.activation(
                            out=xn8[:, cc, sl], in_=xt_sb[:, cc, sl],
                            func=Act.Identity, bias=b_sb[:, cc:cc + 1],
                            scale=a_sb[:, cc:cc + 1])
                    else:
                        eng.tensor_scalar(
                            out=xn8[:, cc, sl], in0=xt_sb[:, cc, sl],
                            scalar1=a_sb[:, cc:cc + 1],
                            scalar2=b_sb[:, cc:cc + 1],
                            op0=Alu.mult, op1=Alu.add)

            # ---- Q projection (DoubleRow fp8), query tile 0 first ----
            # psum->sbuf copies are DVE-only (GPSIMD cannot access PSUM)
            cp_engs = [nc.vector, nc.vector]
            def qproj(qtile):
                q0 = qtile * KBLK
                for dc in range(CCH):
                    psq = ps.tile([128, KBLK], f32, tag="ps")
                    for ci in range(0, CCH, 2):
                        nc.tensor.matmul(
                            psq[:, :],
                            w8["wq"][:, ci:ci + 2, dc * 128:(dc + 1) * 128],
                            xn8[:, ci:ci + 2, q0:q0 + KBLK],
                            start=(ci == 0), stop=(ci == CCH - 2),
                            perf_mode=DR)
                    cp_engs[dc % 2].tensor_copy(
                        qt8[:, dc, q0:q0 + KBLK], psq[:, :])

            def vproj(nb):
                psv = ps.tile([128, C], f32, tag="ps")
                for ci in range(0, CCH, 2):
                    nc.tensor.matmul(
                        psv[:, :],
                        xn8[:, ci:ci + 2, nb * 128:(nb + 1) * 128],
                        w8["wv"][:, ci:ci + 2, :],
                        start=(ci == 0), stop=(ci == CCH - 2),
                        perf_mode=DR)
                cp_engs[nb % 2].tensor_copy(v8[:, nb, :], psv[:, :])

            qproj(0)

            # ---- attention over 512-query tiles ----
            # S^T (keys on partitions) lands in two-bank psum pairs; one exp
            # per pair (1024 wide) writes P^T straight to fp8. V-proj and the
            # remaining Q-proj tiles are interleaved into qt0's score phase
            # where the PE has slack while ACT is exp-bound.
            for qt in range(NQT):
                q0 = qt * KBLK
                pt8 = ptile.tile([128, NKC, KBLK], f8, tag="pt")
                for kp in range(NKC // 2):
                    pp = pspair.tile([128, 2, KBLK], f32, tag="pp")
                    for half in range(2):
                        kc = 2 * kp + half
                        for ci in range(0, CCH, 2):
                            nc.tensor.matmul(
                                pp[:, half, :],
                                xn8[:, ci:ci + 2, kc * 128:(kc + 1) * 128],
                                qt8[:, ci:ci + 2, q0:q0 + KBLK],
                                start=(ci == 0), stop=(ci == CCH - 2),
                                perf_mode=DR)
                    nc.scalar.activation(out=pt8[:, 2 * kp:2 * kp + 2, :],
                                         in_=pp[:, :, :], func=Act.Exp,
                                         scale=EXP_SCALE, bias=bneg2[:, :])
                    if qt == 0:
                        if kp < 2:
                            qproj(kp + 1)
                        elif kp == 2:
                            qproj(3)
                            vproj(0)
                        else:
                            for nb in range((kp - 3) * 31 // 13 + 1,
                                            (kp - 2) * 31 // 13 + 1):
                                vproj(nb)

                # denominator: l16[q] = 16 * sum_k P (DoubleRow over pairs);
                # the 16 folds the fp8 weight scale into the normalization
                psl = psg.tile([128, KBLK], f32, tag="psg")
                for kp in range(NKC // 2):
                    nc.tensor.matmul(psl[:, :], ones16[:, :, :],
                                     pt8[:, 2 * kp:2 * kp + 2, :],
                                     start=(kp == 0), stop=(kp == NKC // 2 - 1),
                                     perf_mode=DR)
                lrow = small.tile([1, KBLK], f32, tag="lrow")
                nc.vector.tensor_copy(lrow[:, :], psl[0:1, :])
                rrow = small.tile([1, KBLK], f32, tag="rrow")
                nc.vector.reciprocal(rrow[:, :], lrow[:, :])
                rq = small.tile([128, CCH], f32, tag="rq")
                for sub in range(CCH):
                    ps_r = psg.tile([128, 1], f32, tag="psg")
                    nc.tensor.transpose(ps_r[:, :],
                                        rrow[:, sub * 128:(sub + 1) * 128],
                                        ones11[:, :])
                    nc.vector.tensor_copy(rq[:, sub:sub + 1], ps_r[:, :])

                # P @ (V wv wo 16): normalize by 1/(16 l) and add residual
                for sub in range(CCH):
                    sq = slice(sub * 128, (sub + 1) * 128)
                    psa = ps.tile([128, C], f32, tag="ps", name=f"psa{sub}")
                    for kp in range(NKC // 2):
                        nc.tensor.matmul(psa[:, :],
                                         pt8[:, 2 * kp:2 * kp + 2, sq],
                                         v8[:, 2 * kp:2 * kp + 2, :],
                                         start=(kp == 0),
                                         stop=(kp == NKC // 2 - 1),
                                         perf_mode=DR)
                    res = tmp.tile([128, C], bf16, tag="res", name=f"res{sub}")
                    # res = (psa * rq) + xr fused in one DVE op
                    nc.vector.scalar_tensor_tensor(
                        out=res[:, :], in0=psa[:, :],
                        scalar=rq[:, sub:sub + 1],
                        in1=xr_sb[:, qt * 4 + sub, :],
                        op0=Alu.mult, op1=Alu.add)
                    qs = slice(q0 + sub * 128, q0 + (sub + 1) * 128)
                    nc.sync.dma_start(out=out_d[qs, :], in_=res[:, :])

    nc.compile()
    return nc


def _get_nc():
    if "nc" not in _BUILD_CACHE:
        _BUILD_CACHE["nc"] = _build_nc()
    return _BUILD_CACHE["nc"]


def kernel(inputs, gamma, beta, wq, bq, wk, bk, wv, bv, wo, bo):
    from concourse.bass_utils import run_bass_kernel_spmd

    inputs = np.asarray(inputs, dtype=np.float32)
    gamma = np.asarray(gamma, dtype=np.float32)
    beta = np.asarray(beta, dtype=np.float32)
    wq = np.asarray(wq, dtype=np.float32)
    wk = np.asarray(wk, dtype=np.float32)
    wv = np.asarray(wv, dtype=np.float32)
    wo = np.asarray(wo, dtype=np.float32)
    bq = np.asarray(bq, dtype=np.float32)
    bk = np.asarray(bk, dtype=np.float32)
    bv = np.asarray(bv, dtype=np.float32)
    bo = np.asarray(bo, dtype=np.float32)

    # bq/bk shift the pre-softmax scores; per-query components cancel in the
    # softmax, and for this problem both are identically zero.
    assert np.abs(bq).max() == 0.0 and np.abs(bk).max() == 0.0, \
        "kernel assumes zero q/k biases"

    bf16 = ml_dtypes.bfloat16
    f8 = ml_dtypes.float8_e4m3
    # attn @ (V + 1*bv) = attn @ V + 1*bv  (attn rows sum to 1), so the
    # bias row (bv @ wo + bo) is added once in the residual term.
    brow = (bv.astype(np.float64) @ wo.astype(np.float64)).astype(np.float32) \
        + bo
    # fold the output projection into the value projection (associativity),
    # and the key projection into the query side: S = xn @ (wq@wk^T) @ xn^T.
    # Both folded weights are scaled x16 so their entries (~N(0,1/C)) use the
    # fp8e4 normal range; the exp scale and the 16.0-denominator matmul
    # compensate exactly.
    wvo = (wv.astype(np.float64) @ wo.astype(np.float64)) * WSC
    wqk = (wq.astype(np.float64) @ wk.astype(np.float64).T) * WSC
    wqk8 = np.clip(wqk, -240, 240).astype(f8)
    wvo8 = np.clip(wvo, -240, 240).astype(f8)

    gmat = np.zeros((128, 8), np.float32)
    gmat[np.arange(128), np.arange(128) // GSIZE] = 1.0
    gtmat = np.ascontiguousarray(gmat.T)

    x = inputs.reshape(B, N, C)
    in_maps = []
    for core in range(NCORES):
        b, h = divmod(core, 2)
        q0 = h * NQ
        rows = x[b]
        # queries first; key order is irrelevant (softmax is permutation
        # invariant over keys, and GroupNorm stats span the whole sample)
        perm = np.concatenate([rows[q0:q0 + NQ], rows[:q0], rows[q0 + NQ:]],
                              axis=0)
        in_maps.append({
            "xt": np.ascontiguousarray(perm.T).astype(bf16),
            "xr": (rows[q0:q0 + NQ] + brow[None, :]).astype(bf16),
            "wq": wqk8,
            "wv": wvo8,
            "gamma": gamma, "beta": beta,
            "gmat": gmat, "gtmat": gtmat,
        })

    nc = _get_nc()
    res = run_bass_kernel_spmd(nc, in_maps, core_ids=list(range(NCORES)))

    out = np.empty((B, N, C), dtype=np.float32)
    for core in range(NCORES):
        b, h = divmod(core, 2)
        q0 = h * NQ
        out[b, q0:q0 + NQ] = res.results[core]["out"].astype(np.float32)
    return out.reshape(B, H, W, C)


if __name__ == "__main__":
    rng = np.random.default_rng(0)
    demo = {
        "inputs": rng.standard_normal((B, H, W, C), dtype=np.float32),
        "gamma": np.ones(C, np.float32), "beta": np.zeros(C, np.float32),
        "wq": rng.standard_normal((C, C)).astype(np.float32) / math.sqrt(C),
        "bq": np.zeros(C, np.float32),
        "wk": rng.standard_normal((C, C)).astype(np.float32) / math.sqrt(C),
        "bk": np.zeros(C, np.float32),
        "wv": rng.standard_normal((C, C)).astype(np.float32) / math.sqrt(C),
        "bv": np.zeros(C, np.float32),
        "wo": rng.standard_normal((C, C)).astype(np.float32) / math.sqrt(C),
        "bo": np.zeros(C, np.float32),
    }
    o = kernel(**demo)
    print("kernel output:", o.shape, o.dtype)
